# revision 21
# baseline (speedup 1.0000x reference)
"""AttnBlock kernel for 8 Trainium2 NeuronCores.

Strategy (zero cross-core communication):
  - x: [4, 256, 48, 48] -> per batch channel-major [256, 2304].
  - Core i handles batch b = i//2, query-token half r = i%2 (1152 tokens).
  - Each core: LN1 -> q/k/v projections (k,v over all 2304 tokens of its
    batch, redundantly with its pair core), flash-style attention for all
    8 heads over its 1152 query tokens, Wo + residual, LN2, FF (exact
    gelu), 1x1 conv + residual. Everything stays channel-major
    ([channel/inner on partitions, tokens on free]) so every matmul
    contracts on the partition dim with no transposes.
  - Softmax denominators via a ones-column appended to v (M=65 o-matmuls).
  - Weights/activations feeding matmuls are bf16 (full PE rate + fast
    weight loads); LayerNorm statistics and the final 1x1 conv stay in
    float32r (TF32-like) for accuracy. PSUM accumulation is fp32.
  - Attention runs kt-outer so each stationary operand (k-tile / v-tile)
    is loaded into the PE array once per pair of query chunks.
  - Softmax denominator reciprocals are computed in a partition-major
    layout (DMA reshape) - a 1-partition DVE reciprocal is ~6 ns/elem.
  - LayerNorm rstd = exp(-0.5*ln(var+eps)) keeps all ACT work in one
    activation-table set (plus one switch to the gelu set).
  - LayerNorm gains are folded into the weights host-side; biases are
    applied on-device as per-partition / broadcast adds.
  - LN1 + QKV projections and the post-attention tail are streamed in
    token chunks so activations are never fully resident in SBUF.
"""

import sys

sys.path.insert(0, "/opt/trn_rl_repo")

import numpy as np

import concourse.bacc as bacc
import concourse.bass as bass
import concourse.tile as tile
from concourse import mybir
from concourse.bass_utils import run_bass_kernel_spmd

F32 = mybir.dt.float32
F32R = mybir.dt.float32r
BF16 = mybir.dt.bfloat16
ACT_F = mybir.ActivationFunctionType
ALU = mybir.AluOpType

B, C, HH, WW = 4, 256, 48, 48
N = HH * WW            # 2304 tokens per batch
NQ = N // 2            # 1152 query tokens per core
INNER = 512
HEADS = 8
D = 64
CT = C // 128          # 2 channel partition-tiles
MT = INNER // 128      # 4 inner partition-tiles
KT = N // 128          # 18 key-token tiles
SCALE = D ** -0.5
EPS = 1e-5

CH_N = [(0, 512), (512, 512), (1024, 512), (1536, 512), (2048, 256)]
CH_NQ = [(0, 448), (448, 448), (896, 256)]
ATTN_QC = [(0, 512), (512, 512), (1024, 128)]

_cached = {}


def _patch_ldw_opt():
    from concourse import bass_utils
    if getattr(bass_utils, "_ldw_opt_patched", False):
        return
    orig = bass_utils.run_command

    def run2(argv, **kw):
        argv = ["--enable-ldw-opt=true" if a == "--enable-ldw-opt=false" else a
                for a in argv]
        return orig(argv, **kw)

    bass_utils.run_command = run2
    bass_utils._ldw_opt_patched = True


def _patch_act_tables():
    import functools
    import concourse.hw_specs as hw_specs
    if getattr(bacc, "_act_tables_patched", False):
        return
    orig = bacc.get_activation_tables

    @functools.cache
    def patched(arch):
        keep = {"natural_log_exp_and_others", "gelu_and_others"}
        return {name: (funcs if name in keep else frozenset())
                for name, funcs in orig(arch).items()}

    bacc.get_activation_tables = patched
    bacc._act_tables_patched = True


def _build():
    _patch_act_tables()
    nc = bacc.Bacc()

    xb = nc.declare_dram_parameter("xb", [C, N], F32R, isOutput=False)
    xq = nc.declare_dram_parameter("xq", [C, NQ], F32R, isOutput=False)
    wq = nc.declare_dram_parameter("wq", [C, INNER], BF16, isOutput=False)
    wk = nc.declare_dram_parameter("wk", [C, INNER], BF16, isOutput=False)
    wv = nc.declare_dram_parameter("wv", [C, INNER], BF16, isOutput=False)
    wo = nc.declare_dram_parameter("wo", [INNER, C], BF16, isOutput=False)
    wf1 = nc.declare_dram_parameter("wf1", [C, INNER], BF16, isOutput=False)
    wf2 = nc.declare_dram_parameter("wf2", [INNER, C], BF16, isOutput=False)
    wp = nc.declare_dram_parameter("wp", [C, C], F32R, isOutput=False)
    # folded per-row biases: w0q/w0k/w0v = b1 @ W, w0f = b2 @ Wf1 + bf1
    w0q = nc.declare_dram_parameter("w0q", [INNER], F32, isOutput=False)
    w0k = nc.declare_dram_parameter("w0k", [INNER], F32, isOutput=False)
    w0v = nc.declare_dram_parameter("w0v", [INNER], F32, isOutput=False)
    w0f = nc.declare_dram_parameter("w0f", [INNER], F32, isOutput=False)
    bo_d = nc.declare_dram_parameter("bo", [C], F32, isOutput=False)
    bf2_d = nc.declare_dram_parameter("bf2", [C], F32, isOutput=False)
    bp_d = nc.declare_dram_parameter("bp", [C], F32, isOutput=False)
    y = nc.declare_dram_parameter("y", [C, NQ], F32, isOutput=True)

    with tile.TileContext(nc) as tc:
        with tc.tile_pool(name="res", bufs=1) as res, \
             tc.tile_pool(name="rows", bufs=2) as rows, \
             tc.tile_pool(name="kqv", bufs=1) as kqv:

            # ---------- persistent small tensors ----------
            xq_t = res.tile([128, CT, NQ], F32R)
            nc.sync.dma_start(out=xq_t, in_=xq.rearrange("(t p) n -> p t n", p=128))
            wo_t = res.tile([128, MT, C], BF16)
            nc.sync.dma_start(out=wo_t, in_=wo.rearrange("(t p) c -> p t c", p=128))
            wf1_t = res.tile([128, CT, INNER], BF16)
            nc.sync.dma_start(out=wf1_t, in_=wf1.rearrange("(t p) i -> p t i", p=128))
            wf2_t = res.tile([128, MT, C], BF16)
            nc.sync.dma_start(out=wf2_t, in_=wf2.rearrange("(t p) c -> p t c", p=128))
            wp_t = res.tile([128, CT, C], F32R)
            nc.sync.dma_start(out=wp_t, in_=wp.rearrange("(t p) c -> p t c", p=128))
            w0q_t = res.tile([128, MT], F32)
            w0k_t = res.tile([128, MT], F32)
            w0f_t = res.tile([128, MT], F32)
            nc.sync.dma_start(out=w0q_t, in_=w0q.rearrange("(t p) -> p t", p=128))
            nc.sync.dma_start(out=w0k_t, in_=w0k.rearrange("(t p) -> p t", p=128))
            nc.sync.dma_start(out=w0f_t, in_=w0f.rearrange("(t p) -> p t", p=128))
            bo_t = res.tile([128, CT], F32)
            bf2_t = res.tile([128, CT], F32)
            bp_t = res.tile([128, CT], F32)
            nc.sync.dma_start(out=bo_t, in_=bo_d.rearrange("(t p) -> p t", p=128))
            nc.sync.dma_start(out=bf2_t, in_=bf2_d.rearrange("(t p) -> p t", p=128))
            nc.sync.dma_start(out=bp_t, in_=bp_d.rearrange("(t p) -> p t", p=128))
            t1_t = res.tile([128, CT, NQ], F32R)

            ones_f = res.tile([128, 1], F32)
            nc.vector.memset(ones_f, 1.0)
            ones1 = res.tile([128, 1], F32R)
            nc.vector.tensor_copy(out=ones1, in_=ones_f)
            eps_t = res.tile([1, 1], F32)
            nc.vector.memset(eps_t, EPS)

            # big attention operands (live through attention)
            kT_t = kqv.tile([128, MT, N], BF16)
            qT_t = kqv.tile([128, MT, NQ], BF16)
            va_t = kqv.tile([128, KT, HEADS, D + 1], BF16)
            vone_f = res.tile([128, KT, HEADS, 1], F32)
            nc.vector.memset(vone_f, 1.0)
            # (copy casts f32 -> bf16)
            nc.vector.tensor_copy(out=va_t[:, :, :, D:D + 1], in_=vone_f)

            # ---------- per-chunk LN stats -> broadcast rstd / mean*rstd ----
            def ln_chunk(x_t, off, w, ps_pool, bc_pool, label):
                s_ps = ps_pool.tile([1, 512], F32, tag="s_ps", name=f"sps_{label}")
                q_ps = ps_pool.tile([1, 512], F32, tag="q_ps", name=f"qps_{label}")
                for ct in range(CT):
                    nc.tensor.matmul(s_ps[:, 0:w], ones1, x_t[:, ct, off:off + w],
                                     start=(ct == 0), stop=(ct == CT - 1))
                for ct in range(CT):
                    sq = bc_pool.tile([128, 512], F32R, tag="sq", name=f"sq_{label}{ct}")
                    nc.vector.tensor_mul(sq[:, 0:w], x_t[:, ct, off:off + w],
                                         x_t[:, ct, off:off + w])
                    nc.tensor.matmul(q_ps[:, 0:w], ones1, sq[:, 0:w],
                                     start=(ct == 0), stop=(ct == CT - 1))
                rm_r = rows.tile([1, 2, 512], F32, tag="rowA", name=f"rm_{label}")
                msq_r = rows.tile([1, 512], F32, tag="rowC", name=f"msq_{label}")
                mean_r = rm_r[:, 1, :]   # mean, later mean*rstd
                var_r = rm_r[:, 0, :]    # var, later rstd
                nc.vector.tensor_single_scalar(mean_r[:, 0:w], s_ps[:, 0:w], 1.0 / C,
                                               ALU.mult)      # mean
                nc.vector.tensor_mul(msq_r[:, 0:w], mean_r[:, 0:w], mean_r[:, 0:w])
                nc.vector.scalar_tensor_tensor(var_r[:, 0:w], q_ps[:, 0:w], 1.0 / C,
                                               msq_r[:, 0:w], op0=ALU.mult,
                                               op1=ALU.subtract)
                nc.scalar.activation(var_r[:, 0:w], var_r[:, 0:w], ACT_F.Ln, bias=eps_t)
                nc.scalar.activation(var_r[:, 0:w], var_r[:, 0:w], ACT_F.Exp,
                                     scale=-0.5)              # rstd
                nc.vector.tensor_mul(mean_r[:, 0:w], mean_r[:, 0:w], var_r[:, 0:w])  # m*rstd
                rm_b = bc_pool.tile([128, 2, 512], F32, tag="rm_b", name=f"rmb_{label}")
                nc.gpsimd.partition_broadcast(
                    rm_b.rearrange("p a b -> p (a b)"),
                    rm_r.rearrange("p a b -> p (a b)"))
                return rm_b[:, 0, :], rm_b[:, 1, :]

            # ---------- streamed LN1 + k/v projections over all 2304 ------
            with tc.tile_pool(name="early", bufs=2) as early, \
                 tc.tile_pool(name="ps_stat", bufs=2, space="PSUM") as ps_stat, \
                 tc.tile_pool(name="ps_proj", bufs=3, space="PSUM") as ps_proj:
                wq_t = early.tile([128, CT, INNER], BF16, bufs=1)
                wk_t = early.tile([128, CT, INNER], BF16, bufs=1)
                wv_t = early.tile([128, CT, INNER], BF16, bufs=1)
                nc.sync.dma_start(out=wq_t, in_=wq.rearrange("(t p) i -> p t i", p=128))
                nc.sync.dma_start(out=wk_t, in_=wk.rearrange("(t p) i -> p t i", p=128))
                nc.sync.dma_start(out=wv_t, in_=wv.rearrange("(t p) i -> p t i", p=128))
                w0v_r = rows.tile([1, INNER], F32, bufs=1)
                nc.sync.dma_start(out=w0v_r, in_=w0v.rearrange("(a i) -> a i", a=1))
                w0v_b = early.tile([128, INNER], F32, bufs=1)
                nc.gpsimd.partition_broadcast(w0v_b, w0v_r)

                for off, w in CH_N:
                    xc = early.tile([128, CT, 512], F32R, tag="xc", name=f"xc{off}", bufs=3)
                    nc.sync.dma_start(
                        out=xc[:, :, 0:w],
                        in_=xb[:, off:off + w].rearrange("(t p) n -> p t n", p=128))
                    r_b, mr_b = ln_chunk(xc, 0, w, ps_stat, early, f"ln1_{off}")
                    tn = early.tile([128, CT, 512], BF16, tag="tn", name=f"tn{off}")
                    for ct in range(CT):
                        nc.vector.tensor_mul(tn[:, ct, 0:w], xc[:, ct, 0:w], r_b[:, 0:w])
                        nc.vector.tensor_sub(tn[:, ct, 0:w], tn[:, ct, 0:w], mr_b[:, 0:w])
                    # k^T chunk
                    for mt in range(MT):
                        kp = ps_proj.tile([128, 512], F32, tag="pp", name=f"kp{mt}_{off}")
                        for ct in range(CT):
                            nc.tensor.matmul(kp[:, 0:w],
                                             wk_t[:, ct, 128 * mt:128 * (mt + 1)],
                                             tn[:, ct, 0:w],
                                             start=(ct == 0), stop=(ct == CT - 1))
                        nc.vector.tensor_scalar(kT_t[:, mt, off:off + w], kp[:, 0:w],
                                                w0k_t[:, mt:mt + 1], None, op0=ALU.add)
                    # v chunk (w/128 token tiles)
                    for j in range(w // 128):
                        nt = off // 128 + j
                        vp = ps_proj.tile([128, 512], F32, tag="pp", name=f"vp{nt}")
                        for ct in range(CT):
                            nc.tensor.matmul(vp,
                                             tn[:, ct, 128 * j:128 * (j + 1)],
                                             wv_t[:, ct, :],
                                             start=(ct == 0), stop=(ct == CT - 1))
                        nc.vector.tensor_tensor(
                            out=va_t[:, nt, :, 0:D],
                            in0=vp.rearrange("p (h d) -> p h d", h=HEADS),
                            in1=w0v_b.rearrange("p (h d) -> p h d", h=HEADS),
                            op=ALU.add)
                # q^T from xq (separate LN over the same tokens)
                for off, w in CH_NQ:
                    tq = early.tile([128, CT, 512], BF16, tag="tn", name=f"tq{off}")
                    r_b, mr_b = ln_chunk(xq_t, off, w, ps_stat, early, f"lnq_{off}")
                    for ct in range(CT):
                        nc.vector.tensor_mul(tq[:, ct, 0:w], xq_t[:, ct, off:off + w],
                                             r_b[:, 0:w])
                        nc.vector.tensor_sub(tq[:, ct, 0:w], tq[:, ct, 0:w], mr_b[:, 0:w])
                    for mt in range(MT):
                        qp = ps_proj.tile([128, 512], F32, tag="pp", name=f"qp{mt}_{off}")
                        for ct in range(CT):
                            nc.tensor.matmul(qp[:, 0:w],
                                             wq_t[:, ct, 128 * mt:128 * (mt + 1)],
                                             tq[:, ct, 0:w],
                                             start=(ct == 0), stop=(ct == CT - 1))
                        nc.vector.tensor_scalar(qT_t[:, mt, off:off + w], qp[:, 0:w],
                                                w0q_t[:, mt:mt + 1], None, op0=ALU.add)

            # ---------- attention ----------
            with tc.tile_pool(name="oTp", bufs=1) as oTp:
                oT_p = [oTp.tile([128, NQ], BF16, name=f"oT{i}")
                        for i in range(MT)]
                with tc.tile_pool(name="ps_sim", bufs=1, space="PSUM") as ps_sim, \
                     tc.tile_pool(name="ps_o", bufs=1, space="PSUM") as ps_o, \
                     tc.tile_pool(name="att", bufs=2) as att:
                    def attn_evac(hp, off, w, o_ps, tagk):
                        # evacuate o-psum to SBUF right away (frees the
                        # accumulator banks so following matmuls don't wait
                        # on the normalization chain), then normalize by the
                        # softmax denominators (row 64) via a partition-major
                        # reciprocal (1-partition DVE reciprocal is ~6ns/elem)
                        o_sb = [att.tile([65, 512], F32, tag=f"osb{ab}",
                                         name=f"osb{ab}_{tagk}")
                                for ab in range(2)]
                        for ab in range(2):
                            nc.vector.tensor_copy(out=o_sb[ab][:, 0:w],
                                                  in_=o_ps[ab][:, 0:w])
                        wd = w // 64
                        dsq = att.tile([128, 8], F32, tag="dsq", name=f"dsq_{tagk}")
                        for ab in range(2):
                            nc.sync.dma_start(out=dsq[64 * ab:64 * (ab + 1), 0:wd],
                                              in_=o_sb[ab][64:65, 0:w])
                        nc.vector.reciprocal(out=dsq[:, 0:wd], in_=dsq[:, 0:wd])
                        dpair = att.tile([1, 2, 512], F32, tag="dpair",
                                         name=f"dp_{tagk}")
                        for ab in range(2):
                            nc.sync.dma_start(out=dpair[:, ab, 0:w],
                                              in_=dsq[64 * ab:64 * (ab + 1), 0:wd])
                        denb = att.tile([64, 2, 512], F32, tag="denb",
                                        name=f"db_{tagk}")
                        nc.gpsimd.partition_broadcast(
                            denb.rearrange("p a b -> p (a b)"),
                            dpair.rearrange("p a b -> p (a b)"))
                        for ab in range(2):
                            nc.vector.tensor_mul(
                                out=oT_p[hp][64 * ab:64 * (ab + 1), off:off + w],
                                in0=o_sb[ab][0:64, 0:w], in1=denb[:, ab, 0:w])

                    QCS = [(0, 512), (512, 512)]
                    for hp in range(MT):
                        # main pass: kt-outer, both 512-wide query chunks
                        # inner, so each stationary operand (kT slice / v
                        # tile) is loaded once and reused across chunks.
                        o_ps = {}
                        for ab in range(2):
                            for qi in range(2):
                                o_ps[(ab, qi)] = ps_o.tile(
                                    [65, 512], F32, tag=f"o{ab}{qi}",
                                    name=f"o{ab}{qi}_{hp}")
                        for kt in range(KT):
                            sims = [ps_sim.tile([128, 2, 512], F32,
                                                tag=f"sim{ab}",
                                                name=f"sim{ab}_{hp}_{kt}")
                                    for ab in range(2)]
                            for ab in range(2):
                                pb = 64 * ab
                                for qi, (off, w) in enumerate(QCS):
                                    nc.tensor.matmul(
                                        sims[ab][:, qi, :],
                                        kT_t[pb:pb + 64, hp,
                                             128 * kt:128 * (kt + 1)],
                                        qT_t[pb:pb + 64, hp, off:off + w],
                                        start=True, stop=True,
                                        tile_position=(pb, 0))
                            for ab in range(2):
                                h = 2 * hp + ab
                                et = att.tile([128, 2, 512], BF16,
                                              tag=f"exp{ab}",
                                              name=f"et{ab}_{hp}_{kt}")
                                nc.scalar.activation(out=et, in_=sims[ab],
                                                     func=ACT_F.Exp, scale=SCALE)
                                for qi in range(2):
                                    nc.tensor.matmul(
                                        o_ps[(ab, qi)][:, :],
                                        va_t[:, kt, h, :], et[:, qi, :],
                                        start=(kt == 0), stop=(kt == KT - 1))
                        for qi, (off, w) in enumerate(QCS):
                            attn_evac(hp, off, w,
                                      [o_ps[(0, qi)], o_ps[(1, qi)]],
                                      f"{hp}_{off}")
                        # tail pass: last 128 query tokens, 8-kt sim groups
                        off, w = 1024, 128
                        o_pt = [ps_o.tile([65, 512], F32, tag=f"o{ab}0",
                                          name=f"ot{ab}_{hp}")
                                for ab in range(2)]
                        for g0 in range(0, KT, 8):
                            kts = range(g0, min(g0 + 8, KT))
                            ng = len(kts)
                            sims = [ps_sim.tile([128, 8, 128], F32,
                                                tag=f"sim{ab}",
                                                name=f"simt{ab}_{hp}_{g0}")
                                    for ab in range(2)]
                            for j, kt in enumerate(kts):
                                for ab in range(2):
                                    pb = 64 * ab
                                    nc.tensor.matmul(
                                        sims[ab][:, j, :],
                                        kT_t[pb:pb + 64, hp,
                                             128 * kt:128 * (kt + 1)],
                                        qT_t[pb:pb + 64, hp, off:off + w],
                                        start=True, stop=True,
                                        tile_position=(pb, 0))
                            for ab in range(2):
                                h = 2 * hp + ab
                                et = att.tile([128, 8, 128], BF16,
                                              tag=f"expt{ab}",
                                              name=f"ett{ab}_{hp}_{g0}")
                                nc.scalar.activation(out=et[:, 0:ng, :],
                                                     in_=sims[ab][:, 0:ng, :],
                                                     func=ACT_F.Exp, scale=SCALE)
                                for j, kt in enumerate(kts):
                                    nc.tensor.matmul(
                                        o_pt[ab][:, 0:w], va_t[:, kt, h, :],
                                        et[:, j, :],
                                        start=(kt == 0), stop=(kt == KT - 1))
                        attn_evac(hp, off, w, o_pt, f"{hp}_t")

                # ---------- Wo + residual ----------
                with tc.tile_pool(name="ps_wo", bufs=3, space="PSUM") as ps_wo:
                    for off, w in CH_NQ:
                        for ct in range(CT):
                            op = ps_wo.tile([128, 512], F32, tag="pp",
                                            name=f"wop{ct}_{off}")
                            for it in range(MT):
                                nc.tensor.matmul(op[:, 0:w],
                                                 wo_t[:, it, 128 * ct:128 * (ct + 1)],
                                                 oT_p[it][:, off:off + w],
                                                 start=(it == 0), stop=(it == MT - 1))
                            nc.vector.scalar_tensor_tensor(
                                out=t1_t[:, ct, off:off + w], in0=op[:, 0:w],
                                scalar=bo_t[:, ct:ct + 1],
                                in1=xq_t[:, ct, off:off + w],
                                op0=ALU.add, op1=ALU.add)

            # ---------- LN2 + FF + conv, streamed per chunk ----------
            with tc.tile_pool(name="post", bufs=2) as post, \
                 tc.tile_pool(name="ps_stat2", bufs=2, space="PSUM") as ps_stat2, \
                 tc.tile_pool(name="ps_post", bufs=3, space="PSUM") as ps_post:
                for off, w in CH_NQ:
                    r_b, mr_b = ln_chunk(t1_t, off, w, ps_stat2, post, f"ln2_{off}")
                    l2 = post.tile([128, CT, 512], BF16, tag="l2", name=f"l2_{off}")
                    for ct in range(CT):
                        nc.vector.tensor_mul(l2[:, ct, 0:w], t1_t[:, ct, off:off + w],
                                             r_b[:, 0:w])
                        nc.vector.tensor_sub(l2[:, ct, 0:w], l2[:, ct, 0:w],
                                             mr_b[:, 0:w])
                    fc = post.tile([128, MT, 512], BF16, tag="fc", name=f"fc_{off}")
                    for ft in range(MT):
                        fp = ps_post.tile([128, 512], F32, tag="pp",
                                          name=f"fp{ft}_{off}")
                        for ct in range(CT):
                            nc.tensor.matmul(fp[:, 0:w],
                                             wf1_t[:, ct, 128 * ft:128 * (ft + 1)],
                                             l2[:, ct, 0:w],
                                             start=(ct == 0), stop=(ct == CT - 1))
                        nc.scalar.activation(out=fc[:, ft, 0:w], in_=fp[:, 0:w],
                                             func=ACT_F.Gelu,
                                             bias=w0f_t[:, ft:ft + 1])
                    for ct in range(CT):
                        gp = ps_post.tile([128, 512], F32, tag="pp",
                                          name=f"gp{ct}_{off}")
                        for ft in range(MT):
                            nc.tensor.matmul(gp[:, 0:w],
                                             wf2_t[:, ft, 128 * ct:128 * (ct + 1)],
                                             fc[:, ft, 0:w],
                                             start=(ft == 0), stop=(ft == MT - 1))
                        nc.vector.scalar_tensor_tensor(
                            out=t1_t[:, ct, off:off + w], in0=gp[:, 0:w],
                            scalar=bf2_t[:, ct:ct + 1],
                            in1=t1_t[:, ct, off:off + w],
                            op0=ALU.add, op1=ALU.add)
                    yc = post.tile([128, CT, 512], F32, tag="yc", name=f"yc_{off}")
                    for co in range(CT):
                        cp = ps_post.tile([128, 512], F32, tag="pp",
                                          name=f"cp{co}_{off}")
                        for ci in range(CT):
                            nc.tensor.matmul(cp[:, 0:w],
                                             wp_t[:, ci, 128 * co:128 * (co + 1)],
                                             t1_t[:, ci, off:off + w],
                                             start=(ci == 0), stop=(ci == CT - 1))
                        nc.vector.scalar_tensor_tensor(
                            out=yc[:, co, 0:w], in0=cp[:, 0:w],
                            scalar=bp_t[:, co:co + 1],
                            in1=xq_t[:, co, off:off + w],
                            op0=ALU.add, op1=ALU.add)
                    nc.sync.dma_start(
                        out=y[:, off:off + w].rearrange("(t p) n -> p t n", p=128),
                        in_=yc[:, :, 0:w])

    nc.finalize()
    return nc


def kernel(x, Wq, Wk, Wv, Wo, bo, g1, b1, g2, b2, Wf1, bf1, Wf2, bf2, Wp, bp,
           _trace=False):
    if "nc" not in _cached:
        _cached["nc"] = _build()
    nc = _cached["nc"]

    x = np.asarray(x, np.float32)
    f32 = lambda a: np.ascontiguousarray(np.asarray(a, np.float32))
    x4 = x.reshape(B, C, N)
    Wqf = f32(np.asarray(Wq) * np.asarray(g1)[:, None])
    Wkf = f32(np.asarray(Wk) * np.asarray(g1)[:, None])
    Wvf = f32(np.asarray(Wv) * np.asarray(g1)[:, None])
    Wf1f = f32(np.asarray(Wf1) * np.asarray(g2)[:, None])
    w0q = f32(np.asarray(b1) @ np.asarray(Wq))
    w0k = f32(np.asarray(b1) @ np.asarray(Wk))
    w0v = f32(np.asarray(b1) @ np.asarray(Wv))
    w0f = f32(np.asarray(b2) @ np.asarray(Wf1) + np.asarray(bf1))

    import ml_dtypes
    bf = lambda a: np.ascontiguousarray(a.astype(ml_dtypes.bfloat16))
    shared = {
        "wq": bf(Wqf), "wk": bf(Wkf), "wv": bf(Wvf), "wo": bf(f32(Wo)),
        "wf1": bf(Wf1f), "wf2": bf(f32(Wf2)), "wp": f32(Wp),
        "w0q": w0q, "w0k": w0k, "w0v": w0v,
        "w0f": w0f, "bo": f32(bo), "bf2": f32(bf2), "bp": f32(bp),
    }
    in_maps = []
    for i in range(8):
        b, r = i // 2, i % 2
        m = dict(shared)
        m["xb"] = f32(x4[b])
        m["xq"] = f32(x4[b][:, r * NQ:(r + 1) * NQ])
        in_maps.append(m)

    res = run_bass_kernel_spmd(nc, in_maps, list(range(8)), trace=_trace)
    out = np.empty((B, C, N), np.float32)
    for i in range(8):
        b, r = i // 2, i % 2
        out[b][:, r * NQ:(r + 1) * NQ] = res.results[i]["y"]
    if _trace:
        kernel.last_results = res
    return out.reshape(B, C, HH, WW)


# revision 22
# speedup vs baseline: 1.1142x; 1.1142x over previous
"""AttnBlock kernel for 8 Trainium2 NeuronCores.

Strategy (zero cross-core communication):
  - x: [4, 256, 48, 48] -> per batch channel-major [256, 2304].
  - Core i handles batch b = i//2, query-token half r = i%2 (1152 tokens).
  - Each core: LN1 -> q/k/v projections (k,v over all 2304 tokens of its
    batch, redundantly with its pair core), flash-style attention for all
    8 heads over its 1152 query tokens, Wo + residual, LN2, FF (exact
    gelu), 1x1 conv + residual. Everything stays channel-major
    ([channel/inner on partitions, tokens on free]) so every matmul
    contracts on the partition dim with no transposes.
  - Softmax denominators via a ones-column appended to v (M=65 o-matmuls).
  - Weights/activations feeding matmuls are bf16 (full PE rate + fast
    weight loads); LayerNorm statistics and the final 1x1 conv stay in
    float32r (TF32-like) for accuracy. PSUM accumulation is fp32.
  - Attention runs kt-outer so each stationary operand (k-tile / v-tile)
    is loaded into the PE array once per pair of query chunks.
  - Softmax denominator reciprocals are computed in a partition-major
    layout (DMA reshape) - a 1-partition DVE reciprocal is ~6 ns/elem.
  - LayerNorm rstd = exp(-0.5*ln(var+eps)) keeps all ACT work in one
    activation-table set (plus one switch to the gelu set).
  - LayerNorm gains are folded into the weights host-side; biases are
    applied on-device as per-partition / broadcast adds.
  - LN1 + QKV projections and the post-attention tail are streamed in
    token chunks so activations are never fully resident in SBUF.
"""

import sys

sys.path.insert(0, "/opt/trn_rl_repo")

import numpy as np

import concourse.bacc as bacc
import concourse.bass as bass
import concourse.tile as tile
from concourse import mybir
from concourse.bass_utils import run_bass_kernel_spmd

F32 = mybir.dt.float32
F32R = mybir.dt.float32r
BF16 = mybir.dt.bfloat16
ACT_F = mybir.ActivationFunctionType
ALU = mybir.AluOpType

B, C, HH, WW = 4, 256, 48, 48
N = HH * WW            # 2304 tokens per batch
NQ = N // 2            # 1152 query tokens per core
INNER = 512
HEADS = 8
D = 64
CT = C // 128          # 2 channel partition-tiles
MT = INNER // 128      # 4 inner partition-tiles
KT = N // 128          # 18 key-token tiles
SCALE = D ** -0.5
EPS = 1e-5

CH_N = [(0, 512), (512, 512), (1024, 512), (1536, 512), (2048, 256)]
CH_NQ = [(0, 448), (448, 448), (896, 256)]
ATTN_QC = [(0, 512), (512, 512), (1024, 128)]

_cached = {}


def _patch_ldw_opt():
    from concourse import bass_utils
    if getattr(bass_utils, "_ldw_opt_patched", False):
        return
    orig = bass_utils.run_command

    def run2(argv, **kw):
        argv = ["--enable-ldw-opt=true" if a == "--enable-ldw-opt=false" else a
                for a in argv]
        return orig(argv, **kw)

    bass_utils.run_command = run2
    bass_utils._ldw_opt_patched = True


def _patch_act_tables():
    import functools
    import concourse.hw_specs as hw_specs
    if getattr(bacc, "_act_tables_patched", False):
        return
    orig = bacc.get_activation_tables

    @functools.cache
    def patched(arch):
        keep = {"natural_log_exp_and_others", "gelu_and_others"}
        return {name: (funcs if name in keep else frozenset())
                for name, funcs in orig(arch).items()}

    bacc.get_activation_tables = patched
    bacc._act_tables_patched = True


def _build():
    _patch_act_tables()
    nc = bacc.Bacc()

    xb = nc.declare_dram_parameter("xb", [C, N], F32R, isOutput=False)
    xq = nc.declare_dram_parameter("xq", [C, NQ], F32R, isOutput=False)
    wq = nc.declare_dram_parameter("wq", [C, INNER], BF16, isOutput=False)
    wk = nc.declare_dram_parameter("wk", [C, INNER], BF16, isOutput=False)
    wv = nc.declare_dram_parameter("wv", [C, INNER], BF16, isOutput=False)
    wo = nc.declare_dram_parameter("wo", [INNER, C], BF16, isOutput=False)
    wf1 = nc.declare_dram_parameter("wf1", [C, INNER], BF16, isOutput=False)
    wf2 = nc.declare_dram_parameter("wf2", [INNER, C], BF16, isOutput=False)
    wp = nc.declare_dram_parameter("wp", [C, C], F32R, isOutput=False)
    # folded per-row biases: w0q/w0k/w0v = b1 @ W, w0f = b2 @ Wf1 + bf1
    w0q = nc.declare_dram_parameter("w0q", [INNER], F32, isOutput=False)
    w0k = nc.declare_dram_parameter("w0k", [INNER], F32, isOutput=False)
    w0v = nc.declare_dram_parameter("w0v", [INNER], F32, isOutput=False)
    w0f = nc.declare_dram_parameter("w0f", [INNER], F32, isOutput=False)
    bo_d = nc.declare_dram_parameter("bo", [C], F32, isOutput=False)
    bf2_d = nc.declare_dram_parameter("bf2", [C], F32, isOutput=False)
    bp_d = nc.declare_dram_parameter("bp", [C], F32, isOutput=False)
    y = nc.declare_dram_parameter("y", [C, NQ], F32, isOutput=True)

    with tile.TileContext(nc) as tc:
        with tc.tile_pool(name="res", bufs=1) as res, \
             tc.tile_pool(name="rows", bufs=3) as rows, \
             tc.tile_pool(name="kqv", bufs=1) as kqv:

            # ---------- persistent small tensors ----------
            xq_t = res.tile([128, CT, NQ], F32R)
            nc.sync.dma_start(out=xq_t, in_=xq.rearrange("(t p) n -> p t n", p=128))
            wo_t = res.tile([128, MT, C], BF16)
            nc.sync.dma_start(out=wo_t, in_=wo.rearrange("(t p) c -> p t c", p=128))
            wf1_t = res.tile([128, CT, INNER], BF16)
            nc.sync.dma_start(out=wf1_t, in_=wf1.rearrange("(t p) i -> p t i", p=128))
            wf2_t = res.tile([128, MT, C], BF16)
            nc.sync.dma_start(out=wf2_t, in_=wf2.rearrange("(t p) c -> p t c", p=128))
            wp_t = res.tile([128, CT, C], F32R)
            nc.sync.dma_start(out=wp_t, in_=wp.rearrange("(t p) c -> p t c", p=128))
            w0q_t = res.tile([128, MT], F32)
            w0k_t = res.tile([128, MT], F32)
            w0f_t = res.tile([128, MT], F32)
            nc.sync.dma_start(out=w0q_t, in_=w0q.rearrange("(t p) -> p t", p=128))
            nc.sync.dma_start(out=w0k_t, in_=w0k.rearrange("(t p) -> p t", p=128))
            nc.sync.dma_start(out=w0f_t, in_=w0f.rearrange("(t p) -> p t", p=128))
            bo_t = res.tile([128, CT], F32)
            bf2_t = res.tile([128, CT], F32)
            bp_t = res.tile([128, CT], F32)
            nc.sync.dma_start(out=bo_t, in_=bo_d.rearrange("(t p) -> p t", p=128))
            nc.sync.dma_start(out=bf2_t, in_=bf2_d.rearrange("(t p) -> p t", p=128))
            nc.sync.dma_start(out=bp_t, in_=bp_d.rearrange("(t p) -> p t", p=128))
            t1_t = res.tile([128, CT, NQ], F32R)

            ones_f = res.tile([128, 1], F32)
            nc.vector.memset(ones_f, 1.0)
            ones1 = res.tile([128, 1], F32R)
            nc.vector.tensor_copy(out=ones1, in_=ones_f)
            eps_t = res.tile([1, 1], F32)
            nc.vector.memset(eps_t, EPS)

            # big attention operands (live through attention)
            kT_t = kqv.tile([128, MT, N], BF16)
            qT_t = kqv.tile([128, MT, NQ], BF16)
            va_t = kqv.tile([128, KT, HEADS, D + 1], BF16)
            vone_f = res.tile([128, KT, HEADS, 1], F32)
            nc.vector.memset(vone_f, 1.0)
            # (copy casts f32 -> bf16)
            nc.vector.tensor_copy(out=va_t[:, :, :, D:D + 1], in_=vone_f)

            # ---------- per-chunk LN stats -> broadcast rstd / mean*rstd ----
            def ln_chunk(x_t, off, w, ps_pool, bc_pool, label):
                s_ps = ps_pool.tile([1, 512], F32, tag="s_ps", name=f"sps_{label}")
                q_ps = ps_pool.tile([1, 512], F32, tag="q_ps", name=f"qps_{label}")
                for ct in range(CT):
                    nc.tensor.matmul(s_ps[:, 0:w], ones1, x_t[:, ct, off:off + w],
                                     start=(ct == 0), stop=(ct == CT - 1))
                for ct in range(CT):
                    sq = bc_pool.tile([128, 512], F32R, tag="sq", name=f"sq_{label}{ct}")
                    nc.vector.tensor_mul(sq[:, 0:w], x_t[:, ct, off:off + w],
                                         x_t[:, ct, off:off + w])
                    nc.tensor.matmul(q_ps[:, 0:w], ones1, sq[:, 0:w],
                                     start=(ct == 0), stop=(ct == CT - 1))
                rm_r = rows.tile([1, 2, 512], F32, tag="rowA", name=f"rm_{label}")
                msq_r = rows.tile([1, 512], F32, tag="rowC", name=f"msq_{label}")
                mean_r = rm_r[:, 1, :]   # mean, later mean*rstd
                var_r = rm_r[:, 0, :]    # var, later rstd
                nc.vector.tensor_single_scalar(mean_r[:, 0:w], s_ps[:, 0:w], 1.0 / C,
                                               ALU.mult)      # mean
                nc.vector.tensor_mul(msq_r[:, 0:w], mean_r[:, 0:w], mean_r[:, 0:w])
                nc.vector.scalar_tensor_tensor(var_r[:, 0:w], q_ps[:, 0:w], 1.0 / C,
                                               msq_r[:, 0:w], op0=ALU.mult,
                                               op1=ALU.subtract)
                nc.scalar.activation(var_r[:, 0:w], var_r[:, 0:w], ACT_F.Ln, bias=eps_t)
                nc.scalar.activation(var_r[:, 0:w], var_r[:, 0:w], ACT_F.Exp,
                                     scale=-0.5)              # rstd
                nc.vector.tensor_mul(mean_r[:, 0:w], mean_r[:, 0:w], var_r[:, 0:w])  # m*rstd
                rm_b = bc_pool.tile([128, 2, 512], F32, tag="rm_b", name=f"rmb_{label}")
                nc.gpsimd.partition_broadcast(
                    rm_b.rearrange("p a b -> p (a b)"),
                    rm_r.rearrange("p a b -> p (a b)"))
                return rm_b[:, 0, :], rm_b[:, 1, :]

            # ---------- streamed LN1 + k/v projections over all 2304 ------
            with tc.tile_pool(name="early", bufs=2) as early, \
                 tc.tile_pool(name="ps_stat", bufs=2, space="PSUM") as ps_stat, \
                 tc.tile_pool(name="ps_proj", bufs=4, space="PSUM") as ps_proj:
                wq_t = early.tile([128, CT, INNER], BF16, bufs=1)
                wk_t = early.tile([128, CT, INNER], BF16, bufs=1)
                wv_t = early.tile([128, CT, INNER], BF16, bufs=1)
                nc.sync.dma_start(out=wq_t, in_=wq.rearrange("(t p) i -> p t i", p=128))
                nc.sync.dma_start(out=wk_t, in_=wk.rearrange("(t p) i -> p t i", p=128))
                nc.sync.dma_start(out=wv_t, in_=wv.rearrange("(t p) i -> p t i", p=128))
                w0v_r = rows.tile([1, INNER], F32, bufs=1)
                nc.sync.dma_start(out=w0v_r, in_=w0v.rearrange("(a i) -> a i", a=1))
                w0v_b = early.tile([128, INNER], F32, bufs=1)
                nc.gpsimd.partition_broadcast(w0v_b, w0v_r)

                for off, w in CH_N:
                    xc = early.tile([128, CT, 512], F32R, tag="xc", name=f"xc{off}", bufs=3)
                    nc.sync.dma_start(
                        out=xc[:, :, 0:w],
                        in_=xb[:, off:off + w].rearrange("(t p) n -> p t n", p=128))
                    r_b, mr_b = ln_chunk(xc, 0, w, ps_stat, early, f"ln1_{off}")
                    tn = early.tile([128, CT, 512], BF16, tag="tn", name=f"tn{off}")
                    for ct in range(CT):
                        nc.vector.tensor_mul(tn[:, ct, 0:w], xc[:, ct, 0:w], r_b[:, 0:w])
                        nc.vector.tensor_sub(tn[:, ct, 0:w], tn[:, ct, 0:w], mr_b[:, 0:w])
                    # k^T chunk
                    for mt in range(MT):
                        kp = ps_proj.tile([128, 512], F32, tag="pp", name=f"kp{mt}_{off}")
                        for ct in range(CT):
                            nc.tensor.matmul(kp[:, 0:w],
                                             wk_t[:, ct, 128 * mt:128 * (mt + 1)],
                                             tn[:, ct, 0:w],
                                             start=(ct == 0), stop=(ct == CT - 1))
                        nc.vector.tensor_scalar(kT_t[:, mt, off:off + w], kp[:, 0:w],
                                                w0k_t[:, mt:mt + 1], None, op0=ALU.add)
                    # v chunk (w/128 token tiles)
                    for j in range(w // 128):
                        nt = off // 128 + j
                        vp = ps_proj.tile([128, 512], F32, tag="pp", name=f"vp{nt}")
                        for ct in range(CT):
                            nc.tensor.matmul(vp,
                                             tn[:, ct, 128 * j:128 * (j + 1)],
                                             wv_t[:, ct, :],
                                             start=(ct == 0), stop=(ct == CT - 1))
                        nc.vector.tensor_tensor(
                            out=va_t[:, nt, :, 0:D],
                            in0=vp.rearrange("p (h d) -> p h d", h=HEADS),
                            in1=w0v_b.rearrange("p (h d) -> p h d", h=HEADS),
                            op=ALU.add)
                # q^T from xq (separate LN over the same tokens)
                for off, w in CH_NQ:
                    tq = early.tile([128, CT, 512], BF16, tag="tn", name=f"tq{off}")
                    r_b, mr_b = ln_chunk(xq_t, off, w, ps_stat, early, f"lnq_{off}")
                    for ct in range(CT):
                        nc.vector.tensor_mul(tq[:, ct, 0:w], xq_t[:, ct, off:off + w],
                                             r_b[:, 0:w])
                        nc.vector.tensor_sub(tq[:, ct, 0:w], tq[:, ct, 0:w], mr_b[:, 0:w])
                    for mt in range(MT):
                        qp = ps_proj.tile([128, 512], F32, tag="pp", name=f"qp{mt}_{off}")
                        for ct in range(CT):
                            nc.tensor.matmul(qp[:, 0:w],
                                             wq_t[:, ct, 128 * mt:128 * (mt + 1)],
                                             tq[:, ct, 0:w],
                                             start=(ct == 0), stop=(ct == CT - 1))
                        nc.vector.tensor_scalar(qT_t[:, mt, off:off + w], qp[:, 0:w],
                                                w0q_t[:, mt:mt + 1], None, op0=ALU.add)

            # ---------- attention ----------
            with tc.tile_pool(name="oTp", bufs=1) as oTp:
                oT_p = [oTp.tile([128, NQ], BF16, name=f"oT{i}")
                        for i in range(MT)]
                with tc.tile_pool(name="ps_sim", bufs=1, space="PSUM") as ps_sim, \
                     tc.tile_pool(name="ps_o", bufs=1, space="PSUM") as ps_o, \
                     tc.tile_pool(name="att", bufs=2) as att:
                    def attn_evac(hp, off, w, o_ps, tagk):
                        # evacuate o-psum to SBUF right away (frees the
                        # accumulator banks so following matmuls don't wait
                        # on the normalization chain), then normalize by the
                        # softmax denominators (row 64) via a partition-major
                        # reciprocal (1-partition DVE reciprocal is ~6ns/elem)
                        o_sb = [att.tile([65, 512], F32, tag=f"osb{ab}",
                                         name=f"osb{ab}_{tagk}")
                                for ab in range(2)]
                        for ab in range(2):
                            nc.vector.tensor_copy(out=o_sb[ab][:, 0:w],
                                                  in_=o_ps[ab][:, 0:w])
                        wd = w // 64
                        dsq = att.tile([128, 8], F32, tag="dsq", name=f"dsq_{tagk}")
                        for ab in range(2):
                            nc.sync.dma_start(out=dsq[64 * ab:64 * (ab + 1), 0:wd],
                                              in_=o_sb[ab][64:65, 0:w])
                        nc.vector.reciprocal(out=dsq[:, 0:wd], in_=dsq[:, 0:wd])
                        dpair = att.tile([1, 2, 512], F32, tag="dpair",
                                         name=f"dp_{tagk}")
                        for ab in range(2):
                            nc.sync.dma_start(out=dpair[:, ab, 0:w],
                                              in_=dsq[64 * ab:64 * (ab + 1), 0:wd])
                        denb = att.tile([64, 2, 512], F32, tag="denb",
                                        name=f"db_{tagk}")
                        nc.gpsimd.partition_broadcast(
                            denb.rearrange("p a b -> p (a b)"),
                            dpair.rearrange("p a b -> p (a b)"))
                        for ab in range(2):
                            nc.vector.tensor_mul(
                                out=oT_p[hp][64 * ab:64 * (ab + 1), off:off + w],
                                in0=o_sb[ab][0:64, 0:w], in1=denb[:, ab, 0:w])

                    QCS = [(0, 512), (512, 512)]
                    for hp in range(MT):
                        # main pass: kt-outer, both 512-wide query chunks
                        # inner, so each stationary operand (kT slice / v
                        # tile) is loaded once and reused across chunks.
                        o_ps = {}
                        for ab in range(2):
                            for qi in range(2):
                                o_ps[(ab, qi)] = ps_o.tile(
                                    [65, 512], F32, tag=f"o{ab}{qi}",
                                    name=f"o{ab}{qi}_{hp}")
                        for kt in range(KT):
                            sims = [ps_sim.tile([128, 2, 512], F32,
                                                tag=f"sim{ab}",
                                                name=f"sim{ab}_{hp}_{kt}")
                                    for ab in range(2)]
                            for ab in range(2):
                                pb = 64 * ab
                                for qi, (off, w) in enumerate(QCS):
                                    nc.tensor.matmul(
                                        sims[ab][:, qi, :],
                                        kT_t[pb:pb + 64, hp,
                                             128 * kt:128 * (kt + 1)],
                                        qT_t[pb:pb + 64, hp, off:off + w],
                                        start=True, stop=True,
                                        tile_position=(pb, 0))
                            for ab in range(2):
                                h = 2 * hp + ab
                                et = att.tile([128, 2, 512], BF16,
                                              tag=f"exp{ab}",
                                              name=f"et{ab}_{hp}_{kt}")
                                nc.scalar.activation(out=et, in_=sims[ab],
                                                     func=ACT_F.Exp, scale=SCALE)
                                for qi in range(2):
                                    nc.tensor.matmul(
                                        o_ps[(ab, qi)][:, :],
                                        va_t[:, kt, h, :], et[:, qi, :],
                                        start=(kt == 0), stop=(kt == KT - 1))
                        for qi, (off, w) in enumerate(QCS):
                            attn_evac(hp, off, w,
                                      [o_ps[(0, qi)], o_ps[(1, qi)]],
                                      f"{hp}_{off}")
                        # tail pass: last 128 query tokens, 8-kt sim groups
                        off, w = 1024, 128
                        o_pt = [ps_o.tile([65, 512], F32, tag=f"o{ab}0",
                                          name=f"ot{ab}_{hp}")
                                for ab in range(2)]
                        for g0 in range(0, KT, 8):
                            kts = range(g0, min(g0 + 8, KT))
                            ng = len(kts)
                            sims = [ps_sim.tile([128, 8, 128], F32,
                                                tag=f"sim{ab}",
                                                name=f"simt{ab}_{hp}_{g0}")
                                    for ab in range(2)]
                            for j, kt in enumerate(kts):
                                for ab in range(2):
                                    pb = 64 * ab
                                    nc.tensor.matmul(
                                        sims[ab][:, j, :],
                                        kT_t[pb:pb + 64, hp,
                                             128 * kt:128 * (kt + 1)],
                                        qT_t[pb:pb + 64, hp, off:off + w],
                                        start=True, stop=True,
                                        tile_position=(pb, 0))
                            for ab in range(2):
                                h = 2 * hp + ab
                                et = att.tile([128, 8, 128], BF16,
                                              tag=f"expt{ab}",
                                              name=f"ett{ab}_{hp}_{g0}")
                                nc.scalar.activation(out=et[:, 0:ng, :],
                                                     in_=sims[ab][:, 0:ng, :],
                                                     func=ACT_F.Exp, scale=SCALE)
                                for j, kt in enumerate(kts):
                                    nc.tensor.matmul(
                                        o_pt[ab][:, 0:w], va_t[:, kt, h, :],
                                        et[:, j, :],
                                        start=(kt == 0), stop=(kt == KT - 1))
                        attn_evac(hp, off, w, o_pt, f"{hp}_t")

                # ---------- Wo + residual ----------
                with tc.tile_pool(name="ps_wo", bufs=3, space="PSUM") as ps_wo:
                    for off, w in CH_NQ:
                        for ct in range(CT):
                            op = ps_wo.tile([128, 512], F32, tag="pp",
                                            name=f"wop{ct}_{off}")
                            for it in range(MT):
                                nc.tensor.matmul(op[:, 0:w],
                                                 wo_t[:, it, 128 * ct:128 * (ct + 1)],
                                                 oT_p[it][:, off:off + w],
                                                 start=(it == 0), stop=(it == MT - 1))
                            nc.vector.scalar_tensor_tensor(
                                out=t1_t[:, ct, off:off + w], in0=op[:, 0:w],
                                scalar=bo_t[:, ct:ct + 1],
                                in1=xq_t[:, ct, off:off + w],
                                op0=ALU.add, op1=ALU.add)

            # ---------- LN2 + FF + conv, streamed per chunk ----------
            with tc.tile_pool(name="post", bufs=2) as post, \
                 tc.tile_pool(name="ps_stat2", bufs=2, space="PSUM") as ps_stat2, \
                 tc.tile_pool(name="ps_post", bufs=3, space="PSUM") as ps_post:
                for off, w in CH_NQ:
                    r_b, mr_b = ln_chunk(t1_t, off, w, ps_stat2, post, f"ln2_{off}")
                    l2 = post.tile([128, CT, 512], BF16, tag="l2", name=f"l2_{off}")
                    for ct in range(CT):
                        nc.vector.tensor_mul(l2[:, ct, 0:w], t1_t[:, ct, off:off + w],
                                             r_b[:, 0:w])
                        nc.vector.tensor_sub(l2[:, ct, 0:w], l2[:, ct, 0:w],
                                             mr_b[:, 0:w])
                    fc = post.tile([128, MT, 512], BF16, tag="fc", name=f"fc_{off}")
                    for ft in range(MT):
                        fp = ps_post.tile([128, 512], F32, tag="pp",
                                          name=f"fp{ft}_{off}")
                        for ct in range(CT):
                            nc.tensor.matmul(fp[:, 0:w],
                                             wf1_t[:, ct, 128 * ft:128 * (ft + 1)],
                                             l2[:, ct, 0:w],
                                             start=(ct == 0), stop=(ct == CT - 1))
                        nc.scalar.activation(out=fc[:, ft, 0:w], in_=fp[:, 0:w],
                                             func=ACT_F.Gelu,
                                             bias=w0f_t[:, ft:ft + 1])
                    for ct in range(CT):
                        gp = ps_post.tile([128, 512], F32, tag="pp",
                                          name=f"gp{ct}_{off}")
                        for ft in range(MT):
                            nc.tensor.matmul(gp[:, 0:w],
                                             wf2_t[:, ft, 128 * ct:128 * (ct + 1)],
                                             fc[:, ft, 0:w],
                                             start=(ft == 0), stop=(ft == MT - 1))
                        nc.vector.scalar_tensor_tensor(
                            out=t1_t[:, ct, off:off + w], in0=gp[:, 0:w],
                            scalar=bf2_t[:, ct:ct + 1],
                            in1=t1_t[:, ct, off:off + w],
                            op0=ALU.add, op1=ALU.add)
                    yc = post.tile([128, CT, 512], F32, tag="yc", name=f"yc_{off}")
                    for co in range(CT):
                        cp = ps_post.tile([128, 512], F32, tag="pp",
                                          name=f"cp{co}_{off}")
                        for ci in range(CT):
                            nc.tensor.matmul(cp[:, 0:w],
                                             wp_t[:, ci, 128 * co:128 * (co + 1)],
                                             t1_t[:, ci, off:off + w],
                                             start=(ci == 0), stop=(ci == CT - 1))
                        nc.vector.scalar_tensor_tensor(
                            out=yc[:, co, 0:w], in0=cp[:, 0:w],
                            scalar=bp_t[:, co:co + 1],
                            in1=xq_t[:, co, off:off + w],
                            op0=ALU.add, op1=ALU.add)
                    nc.sync.dma_start(
                        out=y[:, off:off + w].rearrange("(t p) n -> p t n", p=128),
                        in_=yc[:, :, 0:w])

    nc.finalize()
    return nc


def kernel(x, Wq, Wk, Wv, Wo, bo, g1, b1, g2, b2, Wf1, bf1, Wf2, bf2, Wp, bp,
           _trace=False):
    if "nc" not in _cached:
        _cached["nc"] = _build()
    nc = _cached["nc"]

    x = np.asarray(x, np.float32)
    f32 = lambda a: np.ascontiguousarray(np.asarray(a, np.float32))
    x4 = x.reshape(B, C, N)
    Wqf = f32(np.asarray(Wq) * np.asarray(g1)[:, None])
    Wkf = f32(np.asarray(Wk) * np.asarray(g1)[:, None])
    Wvf = f32(np.asarray(Wv) * np.asarray(g1)[:, None])
    Wf1f = f32(np.asarray(Wf1) * np.asarray(g2)[:, None])
    w0q = f32(np.asarray(b1) @ np.asarray(Wq))
    w0k = f32(np.asarray(b1) @ np.asarray(Wk))
    w0v = f32(np.asarray(b1) @ np.asarray(Wv))
    w0f = f32(np.asarray(b2) @ np.asarray(Wf1) + np.asarray(bf1))

    import ml_dtypes
    bf = lambda a: np.ascontiguousarray(a.astype(ml_dtypes.bfloat16))
    shared = {
        "wq": bf(Wqf), "wk": bf(Wkf), "wv": bf(Wvf), "wo": bf(f32(Wo)),
        "wf1": bf(Wf1f), "wf2": bf(f32(Wf2)), "wp": f32(Wp),
        "w0q": w0q, "w0k": w0k, "w0v": w0v,
        "w0f": w0f, "bo": f32(bo), "bf2": f32(bf2), "bp": f32(bp),
    }
    in_maps = []
    for i in range(8):
        b, r = i // 2, i % 2
        m = dict(shared)
        m["xb"] = f32(x4[b])
        m["xq"] = f32(x4[b][:, r * NQ:(r + 1) * NQ])
        in_maps.append(m)

    res = run_bass_kernel_spmd(nc, in_maps, list(range(8)), trace=_trace)
    out = np.empty((B, C, N), np.float32)
    for i in range(8):
        b, r = i // 2, i % 2
        out[b][:, r * NQ:(r + 1) * NQ] = res.results[i]["y"]
    if _trace:
        kernel.last_results = res
    return out.reshape(B, C, HH, WW)
